# revision 34
# baseline (speedup 1.0000x reference)
"""Population-attention (weighted softmax over the query dim) on 8 TRN2 cores.

Math per (b, h):
    S[q,k] = Q[q,:].K[k,:] / 8
    e[q,k] = x[q] * exp(S[q,k])            (softmax over q is shift-invariant,
    A[q,k] = e[q,k] / sum_q' e[q',k]        so the reference's max-subtraction
    O[q,d] = sum_k A[q,k] * x[k] * V[k,d]   is dropped; S ~ N(0,1), no overflow)

Device layout (keys on partitions, queries on the free dim):
    ST[k,q] = sum_d (K[k,d]/8).Q[q,d] + 1*loghi[q] + 1*loglo[q]   (PE, contract=66)
    W[k,q]  = exp(ST)                       (ACT, fused accum -> D[k] = sum_q W)
    Gn[k,d] = V[k,d] * x[k] * (1/D[k])      (DVE tensor_scalar, 2 scalars)
    OT[d,q] = sum_k Gn[k,d] * W[k,q]        (PE, PSUM-accumulated over 16 k-chunks)
    O[q,d]  = OT[d,q].T                     (host transpose on unshard)

Sharding: 32 (b,h) pairs, 4 per core, batch-major so each core sees one batch.
"""

import numpy as np
import ml_dtypes

import concourse.bacc as bacc
import concourse.tile as tile
import concourse.mybir as mybir
from concourse.bass_utils import run_bass_kernel_spmd

B, H, L, D = 2, 16, 2048, 64
NCORES = 8
HPC = (B * H) // NCORES  # heads per core = 4
KC = L // 128            # 16 k-chunks of 128 keys
QH = 2                   # stage-1 q halves (ACT free dim = 1024)
BF16 = mybir.dt.bfloat16
F32 = mybir.dt.float32
NPBF16 = ml_dtypes.bfloat16

_prog_cache = {}


def _build_program():
    if "nc" in _prog_cache:
        return _prog_cache["nc"]
    nc = bacc.Bacc("TRN2", target_bir_lowering=False, debug=False, num_devices=NCORES)
    qt = nc.dram_tensor("qt", [HPC, 66, L], BF16, kind="ExternalInput").ap()
    kt = nc.dram_tensor("kt", [HPC, 66, L], BF16, kind="ExternalInput").ap()
    vv = nc.dram_tensor("vv", [HPC, 128, KC * D], F32, kind="ExternalInput").ap()
    xs = nc.dram_tensor("xs", [128, KC], F32, kind="ExternalInput").ap()
    ot = nc.dram_tensor("ot", [HPC, D, L], F32, kind="ExternalOutput").ap()

    DELAY = 13  # stage-2 backlog capacity in k-chunks (PE slack reservoir)

    with tile.TileContext(nc) as tc, \
         tc.tile_pool(name="io", bufs=2) as io, \
         tc.tile_pool(name="vp", bufs=2) as vp, \
         tc.tile_pool(name="xp", bufs=1) as xp, \
         tc.tile_pool(name="wp", bufs=DELAY + 3) as wp, \
         tc.tile_pool(name="dp", bufs=4) as dp, \
         tc.tile_pool(name="gp", bufs=DELAY + 3) as gp, \
         tc.tile_pool(name="outp", bufs=2) as outp, \
         tc.tile_pool(name="psS", bufs=2, space="PSUM") as psS, \
         tc.tile_pool(name="psO", bufs=1, space="PSUM") as psO:

        xs_t = xp.tile([128, KC], F32)
        nc.sync.dma_start(out=xs_t[:], in_=xs[:])

        po = {}

        for j in range(HPC):
            # split + spread loads across the HW-DGE (sync) and SW-DGE
            # (gpsimd) queue families so they run in parallel
            qt_t = io.tile([66, L], BF16, tag="qt")
            nc.sync.dma_start(out=qt_t[:, 0:512], in_=qt[j, :, 0:512])
            nc.sync.dma_start(out=qt_t[:, 512:1024], in_=qt[j, :, 512:1024])
            nc.sync.dma_start(out=qt_t[:, 1024:L], in_=qt[j, :, 1024:L])
            kt_t = io.tile([66, L], BF16, tag="kt")
            nc.sync.dma_start(out=kt_t[:, 0:256], in_=kt[j, :, 0:256])
            nc.sync.dma_start(out=kt_t[:, 256:L], in_=kt[j, :, 256:L])
            v_t = vp.tile([128, KC * D], F32)
            nc.sync.dma_start(out=v_t[:], in_=vv[j])

            po[j] = psO.tile([64, L], F32, tag="po", name=f"po{j}")

            for kc in range(KC):
                w_t = wp.tile([128, L], BF16)
                d2 = dp.tile([128, 2], F32, tag="d2")
                for hf in range(QH):
                    ps = psS.tile([128, 1024], F32)
                    for qq in range(2):
                        nc.tensor.matmul(
                            ps[:, qq * 512:(qq + 1) * 512],
                            lhsT=kt_t[:, kc * 128:(kc + 1) * 128],
                            rhs=qt_t[:, hf * 1024 + qq * 512: hf * 1024 + (qq + 1) * 512],
                            start=True, stop=True,
                        )
                    nc.scalar.activation(
                        out=w_t[:, hf * 1024:(hf + 1) * 1024],
                        in_=ps[:],
                        func=mybir.ActivationFunctionType.Exp,
                        accum_out=d2[:, 0:1] if hf == 0 else None,
                    )
                # D = (ACT-fused half) + (DVE-reduced half): splits the
                # denominator work between ScalarE and VectorE
                nc.vector.reduce_sum(
                    out=d2[:, 1:2], in_=w_t[:, 1024:2048],
                    axis=mybir.AxisListType.X,
                )
                d_t = dp.tile([128, 1], F32, tag="d")
                nc.vector.tensor_add(d_t[:], d2[:, 0:1], d2[:, 1:2])
                r_t = dp.tile([128, 1], F32, tag="r")
                nc.vector.reciprocal(r_t[:], d_t[:])
                g_t = gp.tile([128, D], BF16)
                nc.vector.tensor_scalar(
                    out=g_t[:],
                    in0=v_t[:, kc * D:(kc + 1) * D],
                    scalar1=xs_t[:, kc:kc + 1],
                    scalar2=r_t[:],
                    op0=mybir.AluOpType.mult,
                    op1=mybir.AluOpType.mult,
                )
                # Demote stage-2 so stage-1 (which feeds the bottleneck
                # ScalarE exp stream) always wins the PE; the W-pool slot
                # count bounds the deferred-stage-2 backlog. Last head runs
                # at normal priority so the kernel tail isn't a serial drain.
                with tc.high_priority(-60):
                    for qc in range(4):
                        nc.tensor.matmul(
                            po[j][:, qc * 512:(qc + 1) * 512],
                            lhsT=g_t[:],
                            rhs=w_t[:, qc * 512:(qc + 1) * 512],
                            start=(kc == 0), stop=(kc == KC - 1),
                        )
            o_t = outp.tile([64, L], F32, tag="ot", name=f"otile{j}")
            nc.vector.tensor_copy(o_t[:], po[j][:])
            nc.sync.dma_start(out=ot[j], in_=o_t[:])

    nc.compile()
    _prog_cache["nc"] = nc
    return nc


def _prepare_in_maps(Q, K, V, x):
    Q = np.asarray(Q, dtype=np.float32)
    K = np.asarray(K, dtype=np.float32)
    V = np.asarray(V, dtype=np.float32)
    x = np.asarray(x, dtype=np.float32)

    logx = np.log(np.maximum(x, 1e-38))
    logx = np.maximum(logx, -80.0).astype(np.float32)      # (B, L)
    lhi = logx.astype(NPBF16)                               # (B, L) bf16
    llo = (logx - lhi.astype(np.float32)).astype(NPBF16)    # residual, bf16

    in_maps = []
    for c in range(NCORES):
        qt = np.empty((HPC, 66, L), dtype=NPBF16)
        kt = np.empty((HPC, 66, L), dtype=NPBF16)
        vv = np.empty((HPC, 128, KC * D), dtype=np.float32)
        for j in range(HPC):
            p = c * HPC + j
            b, h = p // H, p % H
            qt[j, :64] = Q[b, h].T.astype(NPBF16)
            qt[j, 64] = lhi[b]
            qt[j, 65] = llo[b]
            kt[j, :64] = (K[b, h].T / 8.0).astype(NPBF16)
            kt[j, 64:] = NPBF16(1.0)
            vv[j] = V[b, h].reshape(KC, 128, D).transpose(1, 0, 2).reshape(128, KC * D)
        b0 = (c * HPC) // H
        xs = np.ascontiguousarray(x[b0].reshape(KC, 128).T)  # (128, KC)
        in_maps.append({"qt": qt, "kt": kt, "vv": vv, "xs": xs})
    return in_maps


def _assemble(results):
    O = np.empty((B, H, L, D), dtype=np.float32)
    for c in range(NCORES):
        ot = results[c]["ot"]  # (HPC, 64, L)
        for j in range(HPC):
            p = c * HPC + j
            b, h = p // H, p % H
            O[b, h] = ot[j].T
    return O


def run(Q, K, V, x, trace=False):
    nc = _build_program()
    in_maps = _prepare_in_maps(Q, K, V, x)
    res = run_bass_kernel_spmd(nc, in_maps, core_ids=list(range(NCORES)), trace=trace)
    return _assemble(res.results), res


def kernel(Q, K, V, x):
    out, _ = run(Q, K, V, x)
    return out


# revision 35
# speedup vs baseline: 1.2132x; 1.2132x over previous
"""Population-attention (weighted softmax over the query dim) on 8 TRN2 cores.

Math per (b, h):
    S[q,k] = Q[q,:].K[k,:] / 8
    e[q,k] = x[q] * exp(S[q,k])            (softmax over q is shift-invariant,
    A[q,k] = e[q,k] / sum_q' e[q',k]        so the reference's max-subtraction
    O[q,d] = sum_k A[q,k] * x[k] * V[k,d]   is dropped; S ~ N(0,1), no overflow)

Device layout (keys on partitions, queries on the free dim):
    ST[k,q] = sum_d (K[k,d]/8).Q[q,d] + 1*loghi[q] + 1*loglo[q]   (PE, contract=66)
    W[k,q]  = exp(ST)                       (ACT, fused accum -> D[k] = sum_q W)
    Gn[k,d] = V[k,d] * x[k] * (1/D[k])      (DVE tensor_scalar, 2 scalars)
    OT[d,q] = sum_k Gn[k,d] * W[k,q]        (PE, PSUM-accumulated over 16 k-chunks)
    O[q,d]  = OT[d,q].T                     (host transpose on unshard)

Sharding: 32 (b,h) pairs, 4 per core, batch-major so each core sees one batch.
"""

import numpy as np
import ml_dtypes

import concourse.bacc as bacc
import concourse.tile as tile
import concourse.mybir as mybir
from concourse.bass_utils import run_bass_kernel_spmd

B, H, L, D = 2, 16, 2048, 64
NCORES = 8
HPC = (B * H) // NCORES  # heads per core = 4
KC = L // 128            # 16 k-chunks of 128 keys
QH = 2                   # stage-1 q halves (ACT free dim = 1024)
BF16 = mybir.dt.bfloat16
F32 = mybir.dt.float32
NPBF16 = ml_dtypes.bfloat16

_prog_cache = {}


def _build_program():
    if "nc" in _prog_cache:
        return _prog_cache["nc"]
    nc = bacc.Bacc("TRN2", target_bir_lowering=False, debug=False, num_devices=NCORES)
    qt = nc.dram_tensor("qt", [HPC, 66, L], BF16, kind="ExternalInput").ap()
    kt = nc.dram_tensor("kt", [HPC, 66, L], BF16, kind="ExternalInput").ap()
    vv = nc.dram_tensor("vv", [HPC, 128, KC * D], F32, kind="ExternalInput").ap()
    xs = nc.dram_tensor("xs", [128, KC], F32, kind="ExternalInput").ap()
    ot = nc.dram_tensor("ot", [HPC, D, L], F32, kind="ExternalOutput").ap()

    DELAY = 13  # stage-2 backlog capacity in k-chunks (PE slack reservoir)

    with tile.TileContext(nc) as tc, \
         tc.tile_pool(name="io", bufs=2) as io, \
         tc.tile_pool(name="vp", bufs=2) as vp, \
         tc.tile_pool(name="xp", bufs=1) as xp, \
         tc.tile_pool(name="wp", bufs=DELAY + 3) as wp, \
         tc.tile_pool(name="dp", bufs=4) as dp, \
         tc.tile_pool(name="gp", bufs=DELAY + 3) as gp, \
         tc.tile_pool(name="outp", bufs=2) as outp, \
         tc.tile_pool(name="psS", bufs=2, space="PSUM") as psS, \
         tc.tile_pool(name="psO", bufs=1, space="PSUM") as psO:

        xs_t = xp.tile([128, KC], F32)
        nc.sync.dma_start(out=xs_t[:], in_=xs[:])

        po = {}

        for j in range(HPC):
            # split + spread loads across the HW-DGE (sync) and SW-DGE
            # (gpsimd) queue families so they run in parallel
            qt_t = io.tile([66, L], BF16, tag="qt")
            nc.sync.dma_start(out=qt_t[:, 0:512], in_=qt[j, :, 0:512])
            nc.sync.dma_start(out=qt_t[:, 512:1024], in_=qt[j, :, 512:1024])
            nc.sync.dma_start(out=qt_t[:, 1024:L], in_=qt[j, :, 1024:L])
            kt_t = io.tile([66, L], BF16, tag="kt")
            nc.sync.dma_start(out=kt_t[:, 0:256], in_=kt[j, :, 0:256])
            nc.sync.dma_start(out=kt_t[:, 256:L], in_=kt[j, :, 256:L])
            v_t = vp.tile([128, KC * D], F32)
            nc.sync.dma_start(out=v_t[:], in_=vv[j])

            po[j] = psO.tile([64, L], F32, tag="po", name=f"po{j}")

            for kc in range(KC):
                w_t = wp.tile([128, L], BF16)
                d2 = dp.tile([128, 2], F32, tag="d2")
                for hf in range(QH):
                    ps = psS.tile([128, 1024], F32)
                    for qq in range(2):
                        nc.tensor.matmul(
                            ps[:, qq * 512:(qq + 1) * 512],
                            lhsT=kt_t[:, kc * 128:(kc + 1) * 128],
                            rhs=qt_t[:, hf * 1024 + qq * 512: hf * 1024 + (qq + 1) * 512],
                            start=True, stop=True,
                        )
                    nc.scalar.activation(
                        out=w_t[:, hf * 1024:(hf + 1) * 1024],
                        in_=ps[:],
                        func=mybir.ActivationFunctionType.Exp,
                        accum_out=d2[:, 0:1] if hf == 0 else None,
                    )
                # D = (ACT-fused half) + (DVE-reduced half): splits the
                # denominator work between ScalarE and VectorE
                nc.vector.reduce_sum(
                    out=d2[:, 1:2], in_=w_t[:, 1024:2048],
                    axis=mybir.AxisListType.X,
                )
                d_t = dp.tile([128, 1], F32, tag="d")
                nc.vector.tensor_add(d_t[:], d2[:, 0:1], d2[:, 1:2])
                r_t = dp.tile([128, 1], F32, tag="r")
                nc.vector.reciprocal(r_t[:], d_t[:])
                g_t = gp.tile([128, D], BF16)
                nc.vector.tensor_scalar(
                    out=g_t[:],
                    in0=v_t[:, kc * D:(kc + 1) * D],
                    scalar1=xs_t[:, kc:kc + 1],
                    scalar2=r_t[:],
                    op0=mybir.AluOpType.mult,
                    op1=mybir.AluOpType.mult,
                )
                # Demote stage-2 by ~10 chunks of program order so stage-1
                # (which feeds the bottleneck ScalarE exp stream) always wins
                # the PE; the W-pool slot count bounds the deferred backlog.
                with tc.high_priority(-160):
                    for qc in range(4):
                        nc.tensor.matmul(
                            po[j][:, qc * 512:(qc + 1) * 512],
                            lhsT=g_t[:],
                            rhs=w_t[:, qc * 512:(qc + 1) * 512],
                            start=(kc == 0), stop=(kc == KC - 1),
                        )
            o_t = outp.tile([64, L], F32, tag="ot", name=f"otile{j}")
            nc.vector.tensor_copy(o_t[:], po[j][:])
            nc.sync.dma_start(out=ot[j], in_=o_t[:])

    nc.compile()
    _prog_cache["nc"] = nc
    return nc


def _prepare_in_maps(Q, K, V, x):
    Q = np.asarray(Q, dtype=np.float32)
    K = np.asarray(K, dtype=np.float32)
    V = np.asarray(V, dtype=np.float32)
    x = np.asarray(x, dtype=np.float32)

    logx = np.log(np.maximum(x, 1e-38))
    logx = np.maximum(logx, -80.0).astype(np.float32)      # (B, L)
    lhi = logx.astype(NPBF16)                               # (B, L) bf16
    llo = (logx - lhi.astype(np.float32)).astype(NPBF16)    # residual, bf16

    in_maps = []
    for c in range(NCORES):
        qt = np.empty((HPC, 66, L), dtype=NPBF16)
        kt = np.empty((HPC, 66, L), dtype=NPBF16)
        vv = np.empty((HPC, 128, KC * D), dtype=np.float32)
        for j in range(HPC):
            p = c * HPC + j
            b, h = p // H, p % H
            qt[j, :64] = Q[b, h].T.astype(NPBF16)
            qt[j, 64] = lhi[b]
            qt[j, 65] = llo[b]
            kt[j, :64] = (K[b, h].T / 8.0).astype(NPBF16)
            kt[j, 64:] = NPBF16(1.0)
            vv[j] = V[b, h].reshape(KC, 128, D).transpose(1, 0, 2).reshape(128, KC * D)
        b0 = (c * HPC) // H
        xs = np.ascontiguousarray(x[b0].reshape(KC, 128).T)  # (128, KC)
        in_maps.append({"qt": qt, "kt": kt, "vv": vv, "xs": xs})
    return in_maps


def _assemble(results):
    O = np.empty((B, H, L, D), dtype=np.float32)
    for c in range(NCORES):
        ot = results[c]["ot"]  # (HPC, 64, L)
        for j in range(HPC):
            p = c * HPC + j
            b, h = p // H, p % H
            O[b, h] = ot[j].T
    return O


def run(Q, K, V, x, trace=False):
    nc = _build_program()
    in_maps = _prepare_in_maps(Q, K, V, x)
    res = run_bass_kernel_spmd(nc, in_maps, core_ids=list(range(NCORES)), trace=trace)
    return _assemble(res.results), res


def kernel(Q, K, V, x):
    out, _ = run(Q, K, V, x)
    return out
